# revision 15
# baseline (speedup 1.0000x reference)
"""Trainium2 Bass kernel for nn_MixedAttention (ConvBERT-style mixed attention).

Sharding: data-parallel over (batch=4) x (seq halves=2) = 8 cores.
Each core computes output rows [j*1024, (j+1)*1024) of batch b, core = 2*b + j.
k/v are computed redundantly on both cores of a batch pair (no collectives).

Per-core layout strategy (all SBUF tiles [<=128 partitions, free]):
  xT (hidden on partitions, seq on free) drives every projection matmul.
  q,k,kc,co,conv_attn live transposed [a, s]; v lives natural [s, a] (for ctx matmul).
  Attention is computed as S.T = kT.T @ qT tiles [sk=128, sq], exp'd on ACT
  (scale=1/8 folded in), and ctx.T accumulated via lhsT=[v_h | ones] so the
  softmax denominator falls out as row 64; normalization happens after a PE
  transpose back to [s, d] layout.
  Conv branch: depthwise conv as 9 shifted per-partition-scalar FMAs on DVE,
  pointwise matmul, dynamic span kernel softmax'd per head via a selector
  matmul, and the windowed einsum as 9 shifted multiply-adds (split across
  DVE and GpSimd) with span weights broadcast across head dims by DMA.

Emission order is tuned so the conv-branch matmuls (emitted after attention)
fill the PE gaps of the ACT-bound attention phase, keeping the PE HAM clock
gate warm.
"""

import sys

for _p in ("/opt/trn_rl_repo",):
    if _p not in sys.path:
        sys.path.insert(0, _p)

import numpy as np
import ml_dtypes

HIDDEN = 768
N_HEADS = 6
HEAD_DIM = 64
ALL_HEAD = 384
K = 9
B, S = 4, 2048
CHUNK = 1024          # seq rows per core
N_CORES = 8
BF16 = ml_dtypes.bfloat16

_COMPILED = {}


def _build_program():
    import concourse.bass as bass
    import concourse.mybir as mybir
    import concourse.tile as tile
    from concourse import bacc
    from concourse.masks import make_identity
    from contextlib import ExitStack

    dt = mybir.dt
    Alu = mybir.AluOpType
    Act = mybir.ActivationFunctionType

    nc = bacc.Bacc("TRN2", target_bir_lowering=False, debug=False)

    # ---------------- DRAM I/O (host pre-laid in SBUF layout) ----------------
    def din(name, shape, dtype=dt.bfloat16):
        return nc.dram_tensor(name, list(shape), dtype, kind="ExternalInput").ap()

    x_full = din("x_full", [128, 6 * S])              # xT full batch-seq [c, s]
    x_loc = din("x_loc", [128, 6 * 1032])             # xT chunk+-4 (padded/local)
    wq = din("wq", [128, 6 * ALL_HEAD])
    wk = din("wk", [128, 6 * ALL_HEAD])
    wv = din("wv", [128, 6 * ALL_HEAD])
    wco = din("wco", [128, 6 * ALL_HEAD])
    wpw = din("wpw", [128, 6 * ALL_HEAD])
    wck = din("wck", [128, 3 * 54])
    dwd = din("dwd", [128, 6 * K * 128])              # diag depthwise mats
    sel = din("sel", [54, 6])                          # head-sum selector
    bvrow = din("bvrow", [1, ALL_HEAD])
    comask = din("comask", [1, 1032])
    bq = din("bq", [128, 3], dt.float32)
    bk = din("bk", [128, 3], dt.float32)
    convb = din("convb", [128, 3], dt.float32)
    bco = din("bco", [128, 3], dt.float32)
    bck = din("bck", [54, 1], dt.float32)

    out = nc.dram_tensor("out", [128, 8 * 768], dt.float32, kind="ExternalOutput").ap()
    pck_dram = nc.dram_tensor("pck_scratch", [54, CHUNK], dt.bfloat16).ap()

    with tile.TileContext(nc) as tc, ExitStack() as ctx:
        singles = ctx.enter_context(tc.tile_pool(name="singles", bufs=1))
        persist = ctx.enter_context(tc.tile_pool(name="persist", bufs=1))
        work = ctx.enter_context(tc.tile_pool(name="work", bufs=3))

        # ---------------- load constants ----------------
        def load(pool, src, shape, dtype=dt.bfloat16, name=None):
            t = pool.tile(shape, dtype, name=name)
            nc.sync.dma_start(out=t, in_=src)
            return t

        xsb = load(singles, x_full, [128, 6, S], name="xsb")
        xlsb = load(singles, x_loc, [128, 6, 1032], name="xlsb")
        wq_sb = load(singles, wq, [128, 6, ALL_HEAD], name="wq_sb")
        wk_sb = load(singles, wk, [128, 6, ALL_HEAD], name="wk_sb")
        wv_sb = load(singles, wv, [128, 6, ALL_HEAD], name="wv_sb")
        wco_sb = load(singles, wco, [128, 6, ALL_HEAD], name="wco_sb")
        wpw_sb = load(singles, wpw, [128, 6, ALL_HEAD], name="wpw_sb")
        wck_sb = load(singles, wck, [128, 3, 54], name="wck_sb")
        dwd_sb = load(singles, dwd, [128, 6, K, 128], name="dwd_sb")
        sel_sb = load(singles, sel, [54, 6], name="sel_sb")
        bv_sb = load(singles, bvrow, [1, ALL_HEAD], name="bv_sb")
        bq_sb = load(singles, bq, [128, 3], dt.float32, name="bq_sb")
        bk_sb = load(singles, bk, [128, 3], dt.float32, name="bk_sb")
        convb_sb = load(singles, convb, [128, 3], dt.float32, name="convb_sb")
        bco_sb = load(singles, bco, [128, 3], dt.float32, name="bco_sb")
        bck_sb = load(singles, bck, [54, 1], dt.float32, name="bck_sb")

        mask_sb = singles.tile([128, 1032], dt.bfloat16, name="mask_sb")
        nc.gpsimd.dma_start(out=mask_sb, in_=comask.to_broadcast([128, 1032]))

        ident = singles.tile([128, 128], dt.bfloat16, name="ident")
        make_identity(nc, ident)
        ones_sb = singles.tile([1, 128], dt.bfloat16, name="ones_sb")
        nc.vector.memset(ones_sb, 1.0)

        # persistent intermediates
        qT = persist.tile([128, 3, CHUNK], dt.bfloat16, name="qT")
        kT = persist.tile([128, 3, S], dt.bfloat16, name="kT")
        dwT = persist.tile([128, 6, CHUNK], dt.bfloat16, name="dwT")
        kcT = persist.tile([128, 3, CHUNK], dt.bfloat16, name="kcT")
        caT = persist.tile([128, 3, CHUNK], dt.bfloat16, name="caT")
        coT = persist.tile([128, 3, 1032], dt.bfloat16, name="coT")
        vsb = persist.tile([128, 16, 6, 65], dt.bfloat16, name="vsb")
        pck = persist.tile([54, CHUNK], dt.bfloat16, name="pck")
        recipc = persist.tile([128, 8, 6], dt.float32, name="recipc")
        accT = persist.tile([128, 3, CHUNK], dt.bfloat16, name="accT")
        stg = persist.tile([128, 8, 768], dt.float32, name="stg")

        # ---------------- phase B1: q, k, v projections ----------------
        with tc.tile_pool(name="psum_b1", bufs=1, space="PSUM") as pb1:
            # q projection (chunk only; x_loc cols 4..1028); 2 concurrent psums
            for at in range(3):
                ps = [pb1.tile([128, 512], dt.float32, tag="pj", bufs=4,
                               name=f"pq{sb}") for sb in range(2)]
                for dh in range(6):
                    for sb in range(2):
                        nc.tensor.matmul(
                            ps[sb], wq_sb[:, dh, at * 128:(at + 1) * 128],
                            xlsb[:, dh, 4 + sb * 512: 4 + (sb + 1) * 512],
                            start=(dh == 0), stop=(dh == 5))
                for sb in range(2):
                    nc.vector.tensor_scalar_add(
                        qT[:, at, sb * 512:(sb + 1) * 512], ps[sb],
                        bq_sb[:, at:at + 1])
            # k projection over full seq; 4 concurrent psums, 1 LDW per (at,dh)
            for at in range(3):
                ps = [pb1.tile([128, 512], dt.float32, tag="pj", bufs=4,
                               name=f"pk{sb}") for sb in range(4)]
                for dh in range(6):
                    for sb in range(4):
                        nc.tensor.matmul(
                            ps[sb], wk_sb[:, dh, at * 128:(at + 1) * 128],
                            xsb[:, dh, sb * 512:(sb + 1) * 512],
                            start=(dh == 0), stop=(dh == 5))
                for sb in range(4):
                    nc.vector.tensor_scalar_add(
                        kT[:, at, sb * 512:(sb + 1) * 512], ps[sb],
                        bk_sb[:, at:at + 1])
            # v projection, natural [s, a] + ones column; bias via rank-1 matmul
            nc.vector.memset(vsb[:, :, :, 64:65], 1.0)
            for st in range(16):
                pv = pb1.tile([128, ALL_HEAD], dt.float32, tag="pv", bufs=2,
                              name="pv")
                for dh in range(6):
                    nc.tensor.matmul(
                        pv, xsb[:, dh, st * 128:(st + 1) * 128],
                        wv_sb[:, dh, :], start=(dh == 0), stop=False)
                nc.tensor.matmul(pv, ones_sb, bv_sb, start=False, stop=True)
                nc.vector.tensor_copy(vsb[:, st, :, 0:64], pv.rearrange(
                    "p (h d) -> p h d", h=6))

        # ---------------- phase D: attention (ACT-bound; PE gaps filled by
        # the conv-branch work emitted below) ----------------
        pa = ctx.enter_context(tc.tile_pool(name="psum_at", bufs=1, space="PSUM"))
        pb2 = pa

        def attention_head(h):
            at, lo = h // 2, (h % 2) * 64
            cps = [pa.tile([65, 512], dt.float32, tag=f"ctx{sb}", bufs=1,
                           name=f"cps{sb}") for sb in range(2)]
            for sk in range(16):
                sc = pa.tile([128, 1024], dt.float32, tag="sc", bufs=3, name="sc")
                for sb in range(2):
                    nc.tensor.matmul(
                        sc[:, sb * 512:(sb + 1) * 512],
                        kT[lo:lo + 64, at, sk * 128:(sk + 1) * 128],
                        qT[lo:lo + 64, at, sb * 512:(sb + 1) * 512],
                        start=True, stop=True)
                pt = work.tile([128, 1024], dt.bfloat16, tag="pt", bufs=3,
                               name="pt")
                nc.scalar.activation(pt, sc, Act.Exp, scale=0.125)
                for sb in range(2):
                    nc.tensor.matmul(
                        cps[sb], vsb[:, sk, h, :],
                        pt[:, sb * 512:(sb + 1) * 512],
                        start=(sk == 0), stop=(sk == 15))
            # evacuate, transpose back to [s, d], normalize into staging
            for sb in range(2):
                cx = work.tile([65, 512], dt.bfloat16, tag="cx", bufs=4,
                               name="cx")
                nc.vector.tensor_copy(cx, cps[sb])
                for s4 in range(4):
                    st = sb * 4 + s4
                    tp = pa.tile([128, 65], dt.bfloat16, tag="sc", bufs=3,
                                 name="tp")
                    nc.tensor.transpose(
                        tp, cx[:, s4 * 128:(s4 + 1) * 128], ident[0:65, 0:65])
                    rcp = work.tile([128, 1], dt.float32, tag="rcp", bufs=4,
                                    name="rcp")
                    nc.vector.reciprocal(rcp, tp[:, 64:65])
                    nc.vector.tensor_scalar_mul(
                        stg[:, st, h * 64:(h + 1) * 64], tp[:, 0:64], rcp)

        attention_head(0)

        # ---------------- phase B2: conv branch (fills D's PE gaps) ----------
        # depthwise conv: 9 shifted diagonal matmuls on PE
        for ct in range(6):
            for sb in range(2):
                pdw = pb2.tile([128, 512], dt.float32, tag="sc", bufs=3,
                               name="pdw")
                for k in range(K):
                    nc.tensor.matmul(
                        pdw, dwd_sb[:, ct, k, :],
                        xlsb[:, ct, k + sb * 512: k + (sb + 1) * 512],
                        start=(k == 0), stop=(k == K - 1))
                nc.vector.tensor_copy(dwT[:, ct, sb * 512:(sb + 1) * 512], pdw)
        # co projection on chunk+-4 (1032 cols), bias + OOB mask on evac
        for at in range(3):
            for (o, w) in ((0, 512), (512, 512), (1024, 8)):
                pco = pb2.tile([128, 512], dt.float32, tag="sc", bufs=3,
                               name="pco")
                for dh in range(6):
                    nc.tensor.matmul(
                        pco[:, :w], wco_sb[:, dh, at * 128:(at + 1) * 128],
                        xlsb[:, dh, o:o + w],
                        start=(dh == 0), stop=(dh == 5))
                nc.vector.scalar_tensor_tensor(
                    out=coT[:, at, o:o + w], in0=pco[:, :w],
                    scalar=bco_sb[:, at:at + 1], in1=mask_sb[:, o:o + w],
                    op0=Alu.add, op1=Alu.mult)
        # pointwise conv: kcT[a,s] = pw @ dw_out.T (+conv_bias)
        for at in range(3):
            for sb in range(2):
                ppw = pb2.tile([128, 512], dt.float32, tag="sc", bufs=3,
                               name="ppw")
                for dh in range(6):
                    nc.tensor.matmul(
                        ppw, wpw_sb[:, dh, at * 128:(at + 1) * 128],
                        dwT[:, dh, sb * 512:(sb + 1) * 512],
                        start=(dh == 0), stop=(dh == 5))
                nc.vector.tensor_scalar_add(
                    kcT[:, at, sb * 512:(sb + 1) * 512], ppw,
                    convb_sb[:, at:at + 1])
        # conv_attn = kc * q (elementwise, transposed layout)
        for at in range(3):
            nc.vector.tensor_mul(caT[:, at, :], kcT[:, at, :], qT[:, at, :])
        # conv kernel layer -> exp (softmax numerator), [54, s] layout
        for sb in range(2):
            pck_ps = pb2.tile([54, 512], dt.float32, tag="sc", bufs=3,
                              name="pck_ps")
            for at in range(3):
                nc.tensor.matmul(
                    pck_ps, wck_sb[:, at, :],
                    caT[:, at, sb * 512:(sb + 1) * 512],
                    start=(at == 0), stop=(at == 2))
            nc.scalar.activation(pck[:, sb * 512:(sb + 1) * 512], pck_ps,
                                 Act.Exp, bias=bck_sb, scale=1.0)
        nc.sync.dma_start(out=pck_dram, in_=pck)
        # per-head softmax denominators, transposed to [s, h] via matmul
        for st in range(8):
            pdn = pb2.tile([128, 6], dt.float32, tag="sc", bufs=3, name="pdn")
            nc.tensor.matmul(
                pdn, pck[:, st * 128:(st + 1) * 128], sel_sb,
                start=True, stop=True)
            nc.vector.reciprocal(recipc[:, st, :], pdn)

        for h in range(1, N_HEADS):
            attention_head(h)

        # ---------------- phase C: conv window einsum ----------------
        for at in range(3):
            eng = nc.vector
            for k in range(K):
                ckb = work.tile([128, CHUNK], dt.bfloat16, tag="ckb", bufs=3,
                                name="ckb")
                for hh in range(2):
                    src = bass.AP(
                        tensor=pck_dram.tensor,
                        offset=(18 * at + 9 * hh + k) * CHUNK,
                        ap=[[0, 64], [1, CHUNK]])
                    nc.sync.dma_start(out=ckb[hh * 64:(hh + 1) * 64], in_=src)
                if k == 0:
                    eng.tensor_mul(accT[:, at, :], ckb, coT[:, at, 0:CHUNK])
                else:
                    tmp = work.tile([128, CHUNK], dt.bfloat16, tag="tmp", bufs=2,
                                    name="tmp")
                    eng.tensor_mul(tmp, ckb, coT[:, at, k:k + CHUNK])
                    eng.tensor_add(accT[:, at, :], accT[:, at, :], tmp)
            for st in range(8):
                tp2 = pb2.tile([128, 128], dt.bfloat16, tag="sc", bufs=3,
                               name="tp2")
                nc.tensor.transpose(
                    tp2, accT[:, at, st * 128:(st + 1) * 128], ident)
                for hh in range(2):
                    h = at * 2 + hh
                    nc.vector.tensor_scalar_mul(
                        stg[:, st, 384 + h * 64: 384 + (h + 1) * 64],
                        tp2[:, hh * 64:(hh + 1) * 64],
                        recipc[:, st, h:h + 1])

        # ---------------- phase E: write out ----------------
        for st in range(8):
            nc.sync.dma_start(out=out[:, st * 768:(st + 1) * 768],
                              in_=stg[:, st, :])

    nc.compile()
    return nc


def _prep_in_maps(inputs):
    x = np.asarray(inputs["x"], np.float32)
    dw = np.asarray(inputs["dw"], np.float32).reshape(HIDDEN, K)

    def sb_layout(wT, ntile):  # [ntile*128, F] -> [128, ntile*F]
        f = wT.shape[1]
        return np.ascontiguousarray(
            wT.reshape(ntile, 128, f).transpose(1, 0, 2).reshape(128, ntile * f))

    def wprep(w):  # [A, HIDDEN] -> bf16 [128, 6*A]
        return sb_layout(np.ascontiguousarray(w.T).astype(BF16), 6)

    com = {
        "wq": wprep(inputs["Wq"]), "wk": wprep(inputs["Wk"]),
        "wv": wprep(inputs["Wv"]), "wco": wprep(inputs["Wco"]),
        "wpw": wprep(inputs["pw"]),
        "wck": sb_layout(np.ascontiguousarray(inputs["Wck"].T).astype(BF16), 3),
        "sel": np.kron(np.eye(N_HEADS), np.ones((K, 1))).astype(BF16),
        "bvrow": inputs["bv"].reshape(1, ALL_HEAD).astype(BF16),
        "bq": np.ascontiguousarray(inputs["bq"].reshape(3, 128).T, np.float32),
        "bk": np.ascontiguousarray(inputs["bk"].reshape(3, 128).T, np.float32),
        "convb": np.ascontiguousarray(
            inputs["conv_bias"].reshape(3, 128).T, np.float32),
        "bco": np.ascontiguousarray(inputs["bco"].reshape(3, 128).T, np.float32),
        "bck": inputs["bck"].reshape(54, 1).astype(np.float32),
    }
    # diagonal depthwise matrices: dwd[c', ct, k, c] = (c'==c) * dw[ct*128+c', k]
    dwdm = np.zeros((128, 6, K, 128), BF16)
    ii = np.arange(128)
    for ct in range(6):
        for k in range(K):
            dwdm[ii, ct, k, ii] = dw[ct * 128 + ii, k].astype(BF16)
    com["dwd"] = dwdm.reshape(128, 6 * K * 128)

    in_maps = []
    for b in range(B):
        xb = x[b]                                   # [S, HIDDEN]
        xTb = np.ascontiguousarray(xb.T).astype(BF16)   # [768, S]
        xT_pad = np.zeros((HIDDEN, S + 8), BF16)
        xT_pad[:, 4:4 + S] = xTb
        for j in range(2):
            loc = np.ascontiguousarray(xT_pad[:, j * CHUNK: j * CHUNK + 1032])
            g0 = j * CHUNK - 4
            mrows = np.arange(g0, g0 + 1032)
            comask = ((mrows >= 0) & (mrows < S)).astype(BF16).reshape(1, 1032)
            m = dict(com)
            m["x_full"] = sb_layout(xTb, 6)
            m["x_loc"] = sb_layout(loc, 6)
            m["comask"] = comask
            in_maps.append(m)
    return in_maps


def _gather(results):
    # per-core out: [128, 8*768] where row s_local = st*128 + p
    outs = []
    for r in results:
        o = np.asarray(r["out"], np.float32).reshape(128, 8, 768)
        outs.append(np.ascontiguousarray(o.transpose(1, 0, 2)).reshape(1024, 768))
    full = np.stack(outs).reshape(B, 2, CHUNK, 768).reshape(B, S, 768)
    return full


def kernel(**inputs):
    from concourse.bass_utils import run_bass_kernel_spmd

    key = "prog"
    if key not in _COMPILED:
        _COMPILED[key] = _build_program()
    nc = _COMPILED[key]
    in_maps = _prep_in_maps(inputs)
    res = run_bass_kernel_spmd(nc, in_maps, list(range(N_CORES)))
    return _gather(res.results)


if __name__ == "__main__":
    import reference
    inp = {k: np.asarray(v) for k, v in reference.setup_inputs().items()}
    got = kernel(**inp)
    want = np.asarray(reference.reference(**inp))
    err = np.linalg.norm(got - want) / np.linalg.norm(want)
    print("rel err:", err)


# revision 16
# speedup vs baseline: 1.2494x; 1.2494x over previous
"""Trainium2 Bass kernel for nn_MixedAttention (ConvBERT-style mixed attention).

Sharding: data-parallel over (batch=4) x (seq halves=2) = 8 cores.
Each core computes output rows [j*1024, (j+1)*1024) of batch b, core = 2*b + j.
k/v are computed redundantly on both cores of a batch pair (no collectives).

Per-core layout strategy (all SBUF tiles [<=128 partitions, free]):
  xT (hidden on partitions, seq on free) drives every projection matmul.
  q,k,kc,co,conv_attn live transposed [a, s]; v lives natural [s, a] (for ctx matmul).
  Attention is computed as S.T = kT.T @ qT tiles [sk=128, sq], exp'd on ACT
  (scale=1/8 folded in), and ctx.T accumulated via lhsT=[v_h | ones] so the
  softmax denominator falls out as row 64; normalization happens after a PE
  transpose back to [s, d] layout.
  Conv branch: depthwise conv as 9 shifted per-partition-scalar FMAs on DVE,
  pointwise matmul, dynamic span kernel softmax'd per head via a selector
  matmul, and the windowed einsum as 9 shifted multiply-adds (split across
  DVE and GpSimd) with span weights broadcast across head dims by DMA.

Emission order is tuned so the conv-branch matmuls (emitted after attention)
fill the PE gaps of the ACT-bound attention phase, keeping the PE HAM clock
gate warm.
"""

import sys

for _p in ("/opt/trn_rl_repo",):
    if _p not in sys.path:
        sys.path.insert(0, _p)

import numpy as np
import ml_dtypes

HIDDEN = 768
N_HEADS = 6
HEAD_DIM = 64
ALL_HEAD = 384
K = 9
B, S = 4, 2048
CHUNK = 1024          # seq rows per core
N_CORES = 8
BF16 = ml_dtypes.bfloat16

_COMPILED = {}


def _build_program():
    import concourse.bass as bass
    import concourse.mybir as mybir
    import concourse.tile as tile
    from concourse import bacc
    from concourse.masks import make_identity
    from contextlib import ExitStack

    dt = mybir.dt
    Alu = mybir.AluOpType
    Act = mybir.ActivationFunctionType

    nc = bacc.Bacc("TRN2", target_bir_lowering=False, debug=False)

    # ---------------- DRAM I/O (host pre-laid in SBUF layout) ----------------
    def din(name, shape, dtype=dt.bfloat16):
        return nc.dram_tensor(name, list(shape), dtype, kind="ExternalInput").ap()

    x_full = din("x_full", [128, 6 * S])              # xT full batch-seq [c, s]
    x_loc = din("x_loc", [128, 6 * 1032])             # xT chunk+-4 (padded/local)
    wq = din("wq", [128, 6 * ALL_HEAD])
    wk = din("wk", [128, 6 * ALL_HEAD])
    wv = din("wv", [128, 6 * ALL_HEAD])
    wco = din("wco", [128, 6 * ALL_HEAD])
    wpw = din("wpw", [128, 6 * ALL_HEAD])
    wck = din("wck", [128, 3 * 54])
    dwd = din("dwd", [128, 6 * K * 128])              # diag depthwise mats
    sel = din("sel", [54, 6])                          # head-sum selector
    bvrow = din("bvrow", [1, ALL_HEAD])
    comask = din("comask", [1, 1032])
    bq = din("bq", [128, 3], dt.float32)
    bk = din("bk", [128, 3], dt.float32)
    convb = din("convb", [128, 3], dt.float32)
    bco = din("bco", [128, 3], dt.float32)
    bck = din("bck", [54, 1], dt.float32)

    out = nc.dram_tensor("out", [128, 8 * 768], dt.float32, kind="ExternalOutput").ap()
    pck_dram = nc.dram_tensor("pck_scratch", [54, CHUNK], dt.bfloat16).ap()

    with tile.TileContext(nc) as tc, ExitStack() as ctx:
        singles = ctx.enter_context(tc.tile_pool(name="singles", bufs=1))
        persist = ctx.enter_context(tc.tile_pool(name="persist", bufs=1))
        work = ctx.enter_context(tc.tile_pool(name="work", bufs=3))

        # ---------------- load constants ----------------
        def load(pool, src, shape, dtype=dt.bfloat16, name=None):
            t = pool.tile(shape, dtype, name=name)
            nc.sync.dma_start(out=t, in_=src)
            return t

        xsb = load(singles, x_full, [128, 6, S], name="xsb")
        xlsb = load(singles, x_loc, [128, 6, 1032], name="xlsb")
        wq_sb = load(singles, wq, [128, 6, ALL_HEAD], name="wq_sb")
        wk_sb = load(singles, wk, [128, 6, ALL_HEAD], name="wk_sb")
        wv_sb = load(singles, wv, [128, 6, ALL_HEAD], name="wv_sb")
        wco_sb = load(singles, wco, [128, 6, ALL_HEAD], name="wco_sb")
        wpw_sb = load(singles, wpw, [128, 6, ALL_HEAD], name="wpw_sb")
        wck_sb = load(singles, wck, [128, 3, 54], name="wck_sb")
        dwd_sb = load(singles, dwd, [128, 6, K, 128], name="dwd_sb")
        sel_sb = load(singles, sel, [54, 6], name="sel_sb")
        bv_sb = load(singles, bvrow, [1, ALL_HEAD], name="bv_sb")
        bq_sb = load(singles, bq, [128, 3], dt.float32, name="bq_sb")
        bk_sb = load(singles, bk, [128, 3], dt.float32, name="bk_sb")
        convb_sb = load(singles, convb, [128, 3], dt.float32, name="convb_sb")
        bco_sb = load(singles, bco, [128, 3], dt.float32, name="bco_sb")
        bck_sb = load(singles, bck, [54, 1], dt.float32, name="bck_sb")

        mask_sb = singles.tile([128, 1032], dt.bfloat16, name="mask_sb")
        nc.gpsimd.dma_start(out=mask_sb, in_=comask.to_broadcast([128, 1032]))

        ident = singles.tile([128, 128], dt.bfloat16, name="ident")
        make_identity(nc, ident)
        ones_sb = singles.tile([1, 128], dt.bfloat16, name="ones_sb")
        nc.vector.memset(ones_sb, 1.0)

        # persistent intermediates
        qT = persist.tile([128, 3, CHUNK], dt.bfloat16, name="qT")
        kT = persist.tile([128, 3, S], dt.bfloat16, name="kT")
        dwT = persist.tile([128, 6, CHUNK], dt.bfloat16, name="dwT")
        kcT = persist.tile([128, 3, CHUNK], dt.bfloat16, name="kcT")
        caT = persist.tile([128, 3, CHUNK], dt.bfloat16, name="caT")
        coT = persist.tile([128, 3, 1032], dt.bfloat16, name="coT")
        vsb = persist.tile([128, 16, 6, 65], dt.bfloat16, name="vsb")
        pck = persist.tile([54, CHUNK], dt.bfloat16, name="pck")
        recipc = persist.tile([128, 8, 6], dt.float32, name="recipc")
        accT = persist.tile([128, 3, CHUNK], dt.bfloat16, name="accT")
        stg = persist.tile([128, 8, 768], dt.float32, name="stg")

        # ---------------- phase B1: q, k, v projections ----------------
        with tc.tile_pool(name="psum_b1", bufs=1, space="PSUM") as pb1:
            # q projection (chunk only; x_loc cols 4..1028); 2 concurrent psums
            for at in range(3):
                ps = [pb1.tile([128, 512], dt.float32, tag="pj", bufs=4,
                               name=f"pq{sb}") for sb in range(2)]
                for dh in range(6):
                    for sb in range(2):
                        nc.tensor.matmul(
                            ps[sb], wq_sb[:, dh, at * 128:(at + 1) * 128],
                            xlsb[:, dh, 4 + sb * 512: 4 + (sb + 1) * 512],
                            start=(dh == 0), stop=(dh == 5))
                for sb in range(2):
                    nc.vector.tensor_scalar_add(
                        qT[:, at, sb * 512:(sb + 1) * 512], ps[sb],
                        bq_sb[:, at:at + 1])
            # k projection over full seq; 4 concurrent psums, 1 LDW per (at,dh)
            for at in range(3):
                ps = [pb1.tile([128, 512], dt.float32, tag="pj", bufs=4,
                               name=f"pk{sb}") for sb in range(4)]
                for dh in range(6):
                    for sb in range(4):
                        nc.tensor.matmul(
                            ps[sb], wk_sb[:, dh, at * 128:(at + 1) * 128],
                            xsb[:, dh, sb * 512:(sb + 1) * 512],
                            start=(dh == 0), stop=(dh == 5))
                for sb in range(4):
                    nc.vector.tensor_scalar_add(
                        kT[:, at, sb * 512:(sb + 1) * 512], ps[sb],
                        bk_sb[:, at:at + 1])
            # v projection, natural [s, a] + ones column; bias via rank-1 matmul
            nc.vector.memset(vsb[:, :, :, 64:65], 1.0)
            for st in range(16):
                pv = pb1.tile([128, ALL_HEAD], dt.float32, tag="pv", bufs=2,
                              name="pv")
                for dh in range(6):
                    nc.tensor.matmul(
                        pv, xsb[:, dh, st * 128:(st + 1) * 128],
                        wv_sb[:, dh, :], start=(dh == 0), stop=False)
                nc.tensor.matmul(pv, ones_sb, bv_sb, start=False, stop=True)
                nc.vector.tensor_copy(vsb[:, st, :, 0:64], pv.rearrange(
                    "p (h d) -> p h d", h=6))

        # ---------------- phase D: attention (ACT-bound; PE gaps filled by
        # the conv-branch work emitted below) ----------------
        pa = ctx.enter_context(tc.tile_pool(name="psum_at", bufs=1, space="PSUM"))
        pb2 = ctx.enter_context(tc.tile_pool(name="psum_b2", bufs=1, space="PSUM"))

        def attention_head(h):
            at, lo = h // 2, (h % 2) * 64
            cps = [pa.tile([65, 512], dt.float32, tag=f"ctx{sb}", bufs=1,
                           name=f"cps{sb}") for sb in range(2)]
            for sk in range(16):
                sc = pa.tile([128, 1024], dt.float32, tag="sc", bufs=2, name="sc")
                for sb in range(2):
                    nc.tensor.matmul(
                        sc[:, sb * 512:(sb + 1) * 512],
                        kT[lo:lo + 64, at, sk * 128:(sk + 1) * 128],
                        qT[lo:lo + 64, at, sb * 512:(sb + 1) * 512],
                        start=True, stop=True)
                pt = work.tile([128, 1024], dt.bfloat16, tag="pt", bufs=3,
                               name="pt")
                nc.scalar.activation(pt, sc, Act.Exp, scale=0.125)
                for sb in range(2):
                    nc.tensor.matmul(
                        cps[sb], vsb[:, sk, h, :],
                        pt[:, sb * 512:(sb + 1) * 512],
                        start=(sk == 0), stop=(sk == 15))
            # evacuate, transpose back to [s, d], normalize into staging
            for sb in range(2):
                cx = work.tile([65, 512], dt.bfloat16, tag="cx", bufs=4,
                               name="cx")
                nc.vector.tensor_copy(cx, cps[sb])
                for s4 in range(4):
                    st = sb * 4 + s4
                    tp = pa.tile([128, 65], dt.bfloat16, tag="sc", bufs=2,
                                 name="tp")
                    nc.tensor.transpose(
                        tp, cx[:, s4 * 128:(s4 + 1) * 128], ident[0:65, 0:65])
                    rcp = work.tile([128, 1], dt.float32, tag="rcp", bufs=4,
                                    name="rcp")
                    nc.vector.reciprocal(rcp, tp[:, 64:65])
                    nc.vector.tensor_scalar_mul(
                        stg[:, st, h * 64:(h + 1) * 64], tp[:, 0:64], rcp)

        for h in range(N_HEADS):
            attention_head(h)

        # ---------------- phase B2: conv branch (fills D's PE gaps) ----------
        # depthwise conv: 9 shifted diagonal matmuls on PE
        for ct in range(6):
            for sb in range(2):
                pdw = pb2.tile([128, 512], dt.float32, tag="pj", bufs=2,
                               name="pdw")
                for k in range(K):
                    nc.tensor.matmul(
                        pdw, dwd_sb[:, ct, k, :],
                        xlsb[:, ct, k + sb * 512: k + (sb + 1) * 512],
                        start=(k == 0), stop=(k == K - 1))
                nc.vector.tensor_copy(dwT[:, ct, sb * 512:(sb + 1) * 512], pdw)
        # co projection on chunk+-4 (1032 cols), bias + OOB mask on evac
        for at in range(3):
            for (o, w) in ((0, 512), (512, 512), (1024, 8)):
                pco = pb2.tile([128, 512], dt.float32, tag="pj", bufs=2,
                               name="pco")
                for dh in range(6):
                    nc.tensor.matmul(
                        pco[:, :w], wco_sb[:, dh, at * 128:(at + 1) * 128],
                        xlsb[:, dh, o:o + w],
                        start=(dh == 0), stop=(dh == 5))
                nc.vector.scalar_tensor_tensor(
                    out=coT[:, at, o:o + w], in0=pco[:, :w],
                    scalar=bco_sb[:, at:at + 1], in1=mask_sb[:, o:o + w],
                    op0=Alu.add, op1=Alu.mult)
        # pointwise conv: kcT[a,s] = pw @ dw_out.T (+conv_bias)
        for at in range(3):
            for sb in range(2):
                ppw = pb2.tile([128, 512], dt.float32, tag="pj", bufs=2,
                               name="ppw")
                for dh in range(6):
                    nc.tensor.matmul(
                        ppw, wpw_sb[:, dh, at * 128:(at + 1) * 128],
                        dwT[:, dh, sb * 512:(sb + 1) * 512],
                        start=(dh == 0), stop=(dh == 5))
                nc.vector.tensor_scalar_add(
                    kcT[:, at, sb * 512:(sb + 1) * 512], ppw,
                    convb_sb[:, at:at + 1])
        # conv_attn = kc * q (elementwise, transposed layout)
        for at in range(3):
            nc.vector.tensor_mul(caT[:, at, :], kcT[:, at, :], qT[:, at, :])
        # conv kernel layer -> exp (softmax numerator), [54, s] layout
        for sb in range(2):
            pck_ps = pb2.tile([54, 512], dt.float32, tag="pj", bufs=2,
                              name="pck_ps")
            for at in range(3):
                nc.tensor.matmul(
                    pck_ps, wck_sb[:, at, :],
                    caT[:, at, sb * 512:(sb + 1) * 512],
                    start=(at == 0), stop=(at == 2))
            nc.scalar.activation(pck[:, sb * 512:(sb + 1) * 512], pck_ps,
                                 Act.Exp, bias=bck_sb, scale=1.0)
        nc.sync.dma_start(out=pck_dram, in_=pck)
        # per-head softmax denominators, transposed to [s, h] via matmul
        for st in range(8):
            pdn = pb2.tile([128, 6], dt.float32, tag="pj", bufs=2, name="pdn")
            nc.tensor.matmul(
                pdn, pck[:, st * 128:(st + 1) * 128], sel_sb,
                start=True, stop=True)
            nc.vector.reciprocal(recipc[:, st, :], pdn)

        # ---------------- phase C: conv window einsum ----------------
        for at in range(3):
            eng = nc.vector
            for k in range(K):
                ckb = work.tile([128, CHUNK], dt.bfloat16, tag="ckb", bufs=3,
                                name="ckb")
                for hh in range(2):
                    src = bass.AP(
                        tensor=pck_dram.tensor,
                        offset=(18 * at + 9 * hh + k) * CHUNK,
                        ap=[[0, 64], [1, CHUNK]])
                    nc.sync.dma_start(out=ckb[hh * 64:(hh + 1) * 64], in_=src)
                if k == 0:
                    eng.tensor_mul(accT[:, at, :], ckb, coT[:, at, 0:CHUNK])
                else:
                    tmp = work.tile([128, CHUNK], dt.bfloat16, tag="tmp", bufs=2,
                                    name="tmp")
                    eng.tensor_mul(tmp, ckb, coT[:, at, k:k + CHUNK])
                    eng.tensor_add(accT[:, at, :], accT[:, at, :], tmp)
            for st in range(8):
                tp2 = pb2.tile([128, 128], dt.bfloat16, tag="pj", bufs=2,
                               name="tp2")
                nc.tensor.transpose(
                    tp2, accT[:, at, st * 128:(st + 1) * 128], ident)
                for hh in range(2):
                    h = at * 2 + hh
                    nc.vector.tensor_scalar_mul(
                        stg[:, st, 384 + h * 64: 384 + (h + 1) * 64],
                        tp2[:, hh * 64:(hh + 1) * 64],
                        recipc[:, st, h:h + 1])

        # ---------------- phase E: write out ----------------
        for st in range(8):
            nc.sync.dma_start(out=out[:, st * 768:(st + 1) * 768],
                              in_=stg[:, st, :])

    nc.compile()
    return nc


def _prep_in_maps(inputs):
    x = np.asarray(inputs["x"], np.float32)
    dw = np.asarray(inputs["dw"], np.float32).reshape(HIDDEN, K)

    def sb_layout(wT, ntile):  # [ntile*128, F] -> [128, ntile*F]
        f = wT.shape[1]
        return np.ascontiguousarray(
            wT.reshape(ntile, 128, f).transpose(1, 0, 2).reshape(128, ntile * f))

    def wprep(w):  # [A, HIDDEN] -> bf16 [128, 6*A]
        return sb_layout(np.ascontiguousarray(w.T).astype(BF16), 6)

    com = {
        "wq": wprep(inputs["Wq"]), "wk": wprep(inputs["Wk"]),
        "wv": wprep(inputs["Wv"]), "wco": wprep(inputs["Wco"]),
        "wpw": wprep(inputs["pw"]),
        "wck": sb_layout(np.ascontiguousarray(inputs["Wck"].T).astype(BF16), 3),
        "sel": np.kron(np.eye(N_HEADS), np.ones((K, 1))).astype(BF16),
        "bvrow": inputs["bv"].reshape(1, ALL_HEAD).astype(BF16),
        "bq": np.ascontiguousarray(inputs["bq"].reshape(3, 128).T, np.float32),
        "bk": np.ascontiguousarray(inputs["bk"].reshape(3, 128).T, np.float32),
        "convb": np.ascontiguousarray(
            inputs["conv_bias"].reshape(3, 128).T, np.float32),
        "bco": np.ascontiguousarray(inputs["bco"].reshape(3, 128).T, np.float32),
        "bck": inputs["bck"].reshape(54, 1).astype(np.float32),
    }
    # diagonal depthwise matrices: dwd[c', ct, k, c] = (c'==c) * dw[ct*128+c', k]
    dwdm = np.zeros((128, 6, K, 128), BF16)
    ii = np.arange(128)
    for ct in range(6):
        for k in range(K):
            dwdm[ii, ct, k, ii] = dw[ct * 128 + ii, k].astype(BF16)
    com["dwd"] = dwdm.reshape(128, 6 * K * 128)

    in_maps = []
    for b in range(B):
        xb = x[b]                                   # [S, HIDDEN]
        xTb = np.ascontiguousarray(xb.T).astype(BF16)   # [768, S]
        xT_pad = np.zeros((HIDDEN, S + 8), BF16)
        xT_pad[:, 4:4 + S] = xTb
        for j in range(2):
            loc = np.ascontiguousarray(xT_pad[:, j * CHUNK: j * CHUNK + 1032])
            g0 = j * CHUNK - 4
            mrows = np.arange(g0, g0 + 1032)
            comask = ((mrows >= 0) & (mrows < S)).astype(BF16).reshape(1, 1032)
            m = dict(com)
            m["x_full"] = sb_layout(xTb, 6)
            m["x_loc"] = sb_layout(loc, 6)
            m["comask"] = comask
            in_maps.append(m)
    return in_maps


def _gather(results):
    # per-core out: [128, 8*768] where row s_local = st*128 + p
    outs = []
    for r in results:
        o = np.asarray(r["out"], np.float32).reshape(128, 8, 768)
        outs.append(np.ascontiguousarray(o.transpose(1, 0, 2)).reshape(1024, 768))
    full = np.stack(outs).reshape(B, 2, CHUNK, 768).reshape(B, S, 768)
    return full


def kernel(**inputs):
    from concourse.bass_utils import run_bass_kernel_spmd

    key = "prog"
    if key not in _COMPILED:
        _COMPILED[key] = _build_program()
    nc = _COMPILED[key]
    in_maps = _prep_in_maps(inputs)
    res = run_bass_kernel_spmd(nc, in_maps, list(range(N_CORES)))
    return _gather(res.results)


if __name__ == "__main__":
    import reference
    inp = {k: np.asarray(v) for k, v in reference.setup_inputs().items()}
    got = kernel(**inp)
    want = np.asarray(reference.reference(**inp))
    err = np.linalg.norm(got - want) / np.linalg.norm(want)
    print("rel err:", err)
